# revision 1
# baseline (speedup 1.0000x reference)
"""Trainium2 Bass kernel: CenterSurroundConvolution.

out[b,o,h,w] = sum_c center[b,c,h,w]*w_c[c,o] + surround[b,c,h,w]*w_s[c,o] + w_b[o]
where center = I[:,:,1:-1,1:-1], surround = (3x3 box sum of I) - center.

Rewritten as:  out = center @ (w_c - w_s) + box @ w_s + w_b
so only two channel-contraction matmuls are needed per pixel, and both
accumulate into the same PSUM bank.

Mapping (per NeuronCore, data-parallel over batch: 16 images / 8 cores):
  - Input is cast to bf16 on the host (halves input DMA; matmuls are bf16).
  - Each image is processed in horizontal bands (small first/last bands to
    shorten pipeline fill/drain); band input DMAs are prefetched ahead.
  - Vertical 3-tap sum: two row-shifted bf16 tensor_adds merged across the
    two 128-channel chunks (3D APs, 2x DVE mode).
  - Horizontal 3-tap sum: custom DVE op (SLIDE3P), one pass via two
    telescoping prefix scans over a front-zero-padded buffer:
       out[k] = scanA(P[k+2..]) - scanB(P[k..]) + P[k] = P[k]+P[k+1]+P[k+2]
    With P[0:2] == 0 this needs no seed scalars, so the GPSIMD seed chain
    of the earlier version is gone entirely.
  - Channel matmuls in bf16 (full-rate PE), K = 2x128 chunks, M = 2x128,
    N = 378/504 (3-4 output rows), 4 matmuls accumulating per PSUM bank.
  - ACT evacuates PSUM -> SBUF adding the bias, writing bf16 (halves the
    output DMA); the host casts back to fp32.
"""

import sys

import numpy as np

_TRN_REPO = "/opt/trn_rl_repo"
if _TRN_REPO not in sys.path:
    sys.path.insert(0, _TRN_REPO)

import concourse.bacc as bacc
import concourse.mybir as mybir
from concourse import bass_utils, tile
from concourse.dve_spec import AluOp, Spec, Src0, Src1, lower, scan
import concourse.dve_ops as dve_ops
from concourse.dve_uop import (
    DveOpSpec,
    UopConfig,
    UopDpConfig,
    AluInp,
    DelayInp,
    InpSel,
    OutPath,
    OutSel,
    Trigger,
)

# Problem shape (hardcoded per the task contract).
B, C_IN, C_OUT, H, W = 16, 256, 256, 128, 128
N_CORES = 8
IMG_PER_CORE = B // N_CORES          # 2
HO, WO = H - 2, W - 2                # 126, 126

# Variable band sizes: small first band (pipeline fill) and small last
# band (pipeline drain), 24-row bands in the middle.
BANDS = [3, 24, 24, 24, 24, 20, 7]   # output rows per band (sum = 126)
BAND_MAX = max(BANDS)
L_MAX = BAND_MAX * W                 # 3072
assert sum(BANDS) == HO
KC = C_IN // 128                     # 2 contraction chunks
MC = C_OUT // 128                    # 2 output-channel chunks


def _chunks(rows):
    """Split band rows into matmul chunks of 4 (N=504) and 3 (N=378)."""
    n3 = 0
    while (rows - 3 * n3) % 4 != 0:
        n3 += 1
    return [4] * ((rows - 3 * n3) // 4) + [3] * n3


def _slide3f_ref(in0, in1, s0, s1, imm2):
    p = in0.shape[0]
    P = in0.reshape(p, -1).astype(np.float32) + in1.reshape(p, -1)
    out = P.copy()
    out[:, 1:] += P[:, :-1]
    out[:, 2:] += P[:, :-2]
    return out


def _slide3f_uops_1x():
    """1x fallback for the fused op: out[j] = P[j] + P[j-1] + P[j-2],
    P[j] = in0[j] + in1[j], P[<0] = 0. Previous-element values come from
    delay-lane captures of a memory stage's own flop
    (DelayInp.CURR_ALU_OUT = that stage's value from the previous cycle).
    """
    seed = UopConfig()
    seed.enable_input(InpSel.ZERO, 1)
    seed.datapath_config[0].pass_through_delay(0)
    seed.datapath_config[1].enable_alu(AluOp.BYPASS, AluInp.PREV_DELAY_0)
    seed.datapath_config[1].pass_through_delay(0)
    seed.datapath_config[2].enable_alu(AluOp.BYPASS, AluInp.PREV_DELAY_0)
    seed.repeat_count = 1
    seed.trigger = (Trigger.COUNT, Trigger.NONE, Trigger.NONE)
    seed.next_uop = (1, 0, 0)

    u = UopConfig()
    u.enable_input(InpSel.SRC_0, 0)
    u.enable_input(InpSel.SRC_1, 1)
    u.require_inp0 = 1
    u.require_inp1 = 1
    u.trigger = (Trigger.SRC_TENSOR_DONE, Trigger.NONE, Trigger.NONE)
    u.next_uop = (0, 0, 0)
    dp = u.datapath_config
    # S0: P = t + a
    dp[0].enable_alu(AluOp.ADD, AluInp.PREV_ALU_OUT, AluInp.PREV_DELAY_0)
    # S1: memory stage for P; chain1 <- P(j-1), chain2 <- P(j)
    dp[1].enable_alu(AluOp.BYPASS, AluInp.PREV_ALU_OUT)
    dp[1].enable_delay_from_src(DelayInp.CURR_ALU_OUT, 1)
    dp[1].enable_delay_from_src(DelayInp.PREV_ALU_OUT, 2)
    # S2: memory stage for P(j-1); chain3 <- P(j-2)
    dp[2].enable_alu(AluOp.BYPASS, AluInp.PREV_DELAY_1)
    dp[2].enable_delay_from_src(DelayInp.CURR_ALU_OUT, 3)
    dp[2].pass_through_delay(1, 2)
    # S3: P(j) + P(j-1)
    dp[3].enable_alu(AluOp.ADD, AluInp.PREV_DELAY_2, AluInp.PREV_DELAY_1)
    dp[3].pass_through_delay(3)
    # S4: + P(j-2)
    dp[4].enable_alu(AluOp.ADD, AluInp.PREV_ALU_OUT, AluInp.PREV_DELAY_3)
    for s in range(5, 8):
        dp[s].pass_through_alu()
    u.enable_output(OutSel.ALU_OUT, OutPath.WR0_LO)
    return [seed, u]


def _slide3f_uops_2x():
    """2X_1PORT program for the fused vertical-add + horizontal-3-tap op.

    Per cycle (pair i): t_lo=SRC_0, t_hi=SRC_0_HI, a_lo=SRC_1,
    a_hi=SRC_1_HI;  PL = t_lo+a_lo, PH = t_hi+a_hi, SUMA = PL+PH.
    Emitted one pair late:
        out_lo(i) = box[2i-2] = SUMA(i-1) + PL(i)
        out_hi(i) = box[2i-1] = PH(i-1) + SUMA(i)
    Previous-pair values come from delay captures of the memory stages'
    own flops (CURR_ALU_OUT).

    Input lanes (stock 2x convention): lane0=SRC_0 (stage-0 ALU slot),
    lane1=SRC_1 -> chain0, lane2=SRC_0_HI -> chain1, lane3=SRC_1_HI -> chain2.
    """
    seed = UopConfig()
    seed.enable_input(InpSel.ZERO, 1)            # chain0 = 0
    for s in range(3):
        seed.datapath_config[s].pass_through_delay(0)
    seed.datapath_config[3].enable_alu(AluOp.BYPASS, AluInp.PREV_DELAY_0)
    seed.datapath_config[3].pass_through_delay(0)
    seed.datapath_config[4].enable_alu(AluOp.BYPASS, AluInp.PREV_DELAY_0)
    seed.repeat_count = 1
    seed.trigger = (Trigger.COUNT, Trigger.NONE, Trigger.NONE)
    seed.next_uop = (1, 0, 0)

    u = UopConfig()
    u.enable_input(InpSel.SRC_0, 0)              # t_lo (stage-0 view)
    u.enable_input(InpSel.SRC_1, 1)              # a_lo -> chain0
    u.enable_input(InpSel.SRC_0_HI, 2)           # t_hi -> chain1
    u.enable_input(InpSel.SRC_1_HI, 3)           # a_hi -> chain2
    u.require_inp0 = 1
    u.require_inp1 = 1
    u.trigger = (Trigger.SRC_TENSOR_DONE, Trigger.NONE, Trigger.NONE)
    u.next_uop = (0, 0, 0)
    dp = u.datapath_config
    # S0: PL = t_lo + a_lo
    dp[0].enable_alu(AluOp.ADD, AluInp.PREV_ALU_OUT, AluInp.PREV_DELAY_0)
    dp[0].pass_through_delay(1, 2)
    # S1: PH = t_hi + a_hi ; chain3 <- PL
    dp[1].enable_alu(AluOp.ADD, AluInp.PREV_DELAY_1, AluInp.PREV_DELAY_2)
    dp[1].enable_delay_from_src(DelayInp.PREV_ALU_OUT, 3)
    # S2: SUMA = PH + PL ; chain4 <- PH
    dp[2].enable_alu(AluOp.ADD, AluInp.PREV_ALU_OUT, AluInp.PREV_DELAY_3)
    dp[2].pass_through_delay(3)
    dp[2].enable_delay_from_src(DelayInp.PREV_ALU_OUT, 4)
    # S3: memory stage for SUMA ; chain5 <- SUMA(i-1)
    dp[3].enable_alu(AluOp.BYPASS, AluInp.PREV_ALU_OUT)
    dp[3].enable_delay_from_src(DelayInp.CURR_ALU_OUT, 5)
    dp[3].pass_through_delay(3, 4)
    # S4: memory stage for PH ; chain4 <- PH(i-1) ; chain0 <- SUMA(i)
    dp[4].enable_alu(AluOp.BYPASS, AluInp.PREV_DELAY_4)
    dp[4].enable_delay_from_src(DelayInp.CURR_ALU_OUT, 4)
    dp[4].enable_delay_from_src(DelayInp.PREV_ALU_OUT, 0)
    dp[4].pass_through_delay(3, 5)
    # S5: OL = SUMA(i-1) + PL(i)
    dp[5].enable_alu(AluOp.ADD, AluInp.PREV_DELAY_5, AluInp.PREV_DELAY_3)
    dp[5].pass_through_delay(0, 4)
    # S6: OH = PH(i-1) + SUMA(i) ; chain3 <- OL
    dp[6].enable_alu(AluOp.ADD, AluInp.PREV_DELAY_4, AluInp.PREV_DELAY_0)
    dp[6].enable_delay_from_src(DelayInp.PREV_ALU_OUT, 3)
    # S7: bypass OH; OL rides chain3 to the output mux
    dp[7].pass_through_alu()
    dp[7].pass_through_delay(3)
    u.enable_output(OutSel.DELAY_3, OutPath.WR0_LO)
    u.enable_output(OutSel.ALU_OUT, OutPath.WR0_HI)
    return [seed, u]


_SLIDE3 = None


def _get_slide3():
    """Register (once) the fused vertical-add + 3-tap custom DVE op with
    hand-written 1x and 2X_1PORT programs (lower() cannot express the
    cross-element delay captures, so both table programs are authored by
    hand and installed via the compile cache; the Spec body is only a
    placeholder for table registration)."""
    global _SLIDE3
    if _SLIDE3 is not None:
        return _SLIDE3
    for op in dve_ops.OPS:
        if op.name == "SLIDE3F_ANT":
            _SLIDE3 = op
            return op
    spec = Spec(body=Src0 + Src1, reference=_slide3f_ref)
    u1x = _slide3f_uops_1x()
    u2x = _slide3f_uops_2x()
    shas = {}
    specs = {}
    for ver in ("v3", "v4"):
        tmp = DveOpSpec(
            name="SLIDE3F_ANT",
            uops=u1x,
            uops_2x=u2x if ver == "v3" else None,
            rd1_en=True,
            perf_max=1 if ver == "v3" else 0,
        )
        shas[ver] = tmp.sha(ver)
        specs[ver] = tmp
    op = dve_ops.DveOp("SLIDE3F_ANT", spec, subdim=False, uops_sha=shas)
    dve_ops.OPS.append(op)
    dve_ops.CUSTOM_DVE_SPECS[op.name] = spec
    dve_ops._SUB_OPCODE_FOR_NAME[op.name] = dve_ops._CUSTOM_DVE_ROW_BASE + len(
        dve_ops.OPS
    ) - 1
    # Install the hand-written programs: compile() consults this cache, so
    # table-gen writes exactly these uops.
    for ver, tmp in specs.items():
        tmp.opcode = dve_ops.get_dve_sub_opcode(op.name)
        dve_ops._COMPILE_CACHE[(op.name, ver)] = tmp
    _SLIDE3 = op
    return op


def build_module(n_img: int = IMG_PER_CORE):
    slide3 = _get_slide3()
    nc = bacc.Bacc(
        "TRN2", target_bir_lowering=False, debug=False, enable_asserts=False
    )
    f32 = mybir.dt.float32
    bf16 = mybir.dt.bfloat16

    I = nc.dram_tensor("I", [n_img, C_IN, H, W], bf16, kind="ExternalInput").ap()
    wcp = nc.dram_tensor("wcp", [C_IN, C_OUT], bf16, kind="ExternalInput").ap()
    ws = nc.dram_tensor("ws", [C_IN, C_OUT], bf16, kind="ExternalInput").ap()
    wb = nc.dram_tensor("wb", [C_OUT], f32, kind="ExternalInput").ap()
    out = nc.dram_tensor(
        "out", [n_img, C_OUT, HO, WO], bf16, kind="ExternalOutput"
    ).ap()

    with tile.TileContext(nc) as tc:
        with (
            tc.tile_pool(name="wts", bufs=1) as wpool,
            tc.tile_pool(name="io", bufs=4) as iopool,
            tc.tile_pool(name="rs", bufs=1) as rspool,
            tc.tile_pool(name="t1p", bufs=3) as t1pool,
            tc.tile_pool(name="box", bufs=3) as boxpool,
            tc.tile_pool(name="outp", bufs=2) as outpool,
            tc.tile_pool(name="ps", bufs=8, space="PSUM") as pspool,
        ):
            # Stationary weights: [128, w(2), k(2), m*128] (w=0: w_c - w_s, w=1: w_s)
            wt = wpool.tile([128, 2, KC, MC * 128], bf16)
            bias = wpool.tile([128, MC], f32)

            def emit_weight_dma():
                for wi, wsrc in enumerate((wcp, ws)):
                    for k in range(KC):
                        nc.sync.dma_start(
                            wt[:, wi, k, :], wsrc[k * 128 : (k + 1) * 128, :]
                        )
                nc.sync.dma_start(bias[:, :], wb.rearrange("(m p) -> p m", p=128))

            # Interleave the two images' bands so the small fill/drain bands
            # of one image always overlap the big bands of the other --
            # otherwise the PE starves at the image boundary while the DVE
            # rebuilds the next image's first box sums.
            per_img = []
            for b in range(n_img):
                h0 = 0
                row = []
                for band_rows in BANDS:
                    row.append((b, h0, band_rows))
                    h0 += band_rows
                per_img.append(row)
            jobs = [j for tup in zip(*per_img) for j in tup]

            def emit_dma(job):
                b, h0, band_rows = job
                l_in = (band_rows + 2) * W
                Ib = I[b].rearrange("c h w -> c (h w)")
                it = iopool.tile(
                    [128, KC, l_in], bf16, tag="it", name=f"it{b}_{h0}"
                )
                src = Ib.rearrange("(k p) x -> p k x", p=128)[
                    :, :, h0 * W : h0 * W + l_in
                ]
                nc.sync.dma_start(it[:, :, :], src)
                return it

            def emit_compute(job, it, split=False):
                b, h0, band_rows = job
                l_cs = band_rows * W
                Ob = out[b].rearrange("(m p) h w -> p m (h w)", p=128)
                it_rows = it.rearrange("p k (h w) -> p k h w", w=W)
                boxt = boxpool.tile(
                    [128, KC, l_cs + 2], bf16, tag="box", name="boxt"
                )
                # First vertical partial sum t1 = a0 + a2, both k chunks in
                # one op (3D APs). Two tail pad elements are zeroed so the
                # fused op can read l_cs+2 elements.
                t1 = t1pool.tile([128, KC, l_cs + 2], bf16, tag="t1", name="t1")
                nc.vector.tensor_add(
                    t1[:, :, 0:l_cs], it[:, :, 0:l_cs], it[:, :, 2 * W :]
                )
                nc.vector.memset(t1[:, :, l_cs : l_cs + 2], 0.0)
                # Fused op: P = t1 + a1 on the fly, 3-tap FIR over P via
                # previous-element delay captures; box[j] lands at
                # boxt[..., j+2]. The first/last two outputs per k are the
                # (discarded) w=126,127 pad columns, so the k-boundary
                # history carry-over is harmless.
                bi = nc.vector._custom_dve(
                    slide3,
                    out=boxt[:, :, :],
                    in0=t1[:, :, 0 : l_cs + 2],
                    in1=it[:, :, W : W + l_cs + 2],
                )
                bi.ins.perf_max = 1  # allow 2X_1PORT

                ot = outpool.tile(
                    [128, MC, band_rows * WO], bf16, tag="ot", name="ot"
                )
                box_rows = [
                    boxt[:, k, 2 : 2 + l_cs].rearrange("p (h w) -> p h w", w=W)
                    for k in range(KC)
                ]
                def mm_quads(ps, m, r0, crows, quads, start, stop):
                    nmm = crows * WO
                    for qi, (wi, k) in enumerate(quads):
                        lhsT = wt[:, wi, k, m * 128 : (m + 1) * 128]
                        if wi == 0:
                            rhs = it_rows[
                                :, k, 1 + r0 : 1 + r0 + crows, 1 : 1 + WO
                            ]
                        else:
                            rhs = box_rows[k][:, r0 : r0 + crows, 0:WO]
                        nc.tensor.matmul(
                            ps[:, 0:nmm], lhsT, rhs,
                            start=(start and qi == 0),
                            stop=(stop and qi == len(quads) - 1),
                        )

                if split:
                    # Pipeline-fill bands: emit the center matmuls of every
                    # chunk first (they depend only on the input DMA and the
                    # weights, so they both warm the HAM clock and do real
                    # work while the DVE computes this band's box), then the
                    # box matmuls + evacuation.
                    held = []
                    for m in range(MC):
                        r0 = 0
                        for crows in _chunks(band_rows):
                            ps = pspool.tile(
                                [128, 512], f32, tag="ps", name="ps"
                            )
                            mm_quads(ps, m, r0, crows, [(0, 0), (0, 1)],
                                     True, False)
                            held.append((m, r0, crows, ps))
                            r0 += crows
                    last_m = -1
                    for m, r0, crows, ps in held:
                        nmm = crows * WO
                        mm_quads(ps, m, r0, crows, [(1, 0), (1, 1)],
                                 False, True)
                        nc.scalar.activation(
                            ot[:, m, r0 * WO : r0 * WO + nmm],
                            ps[:, 0:nmm],
                            mybir.ActivationFunctionType.Identity,
                            bias=bias[:, m : m + 1],
                        )
                    for m in range(MC):
                        nc.scalar.dma_start(
                            Ob[:, m, h0 * WO : (h0 + band_rows) * WO],
                            ot[:, m, :],
                        )
                    return

                for m in range(MC):
                    r0 = 0
                    for crows in _chunks(band_rows):
                        nmm = crows * WO
                        ps = pspool.tile([128, 512], f32, tag="ps", name="ps")
                        mm_quads(ps, m, r0, crows,
                                 [(0, 0), (0, 1), (1, 0), (1, 1)], True, True)
                        nc.scalar.activation(
                            ot[:, m, r0 * WO : r0 * WO + nmm],
                            ps[:, 0:nmm],
                            mybir.ActivationFunctionType.Identity,
                            bias=bias[:, m : m + 1],
                        )
                        r0 += crows
                    # drain this m-half as soon as its last ACT finishes;
                    # issued from the ACT engine itself (in-order, no
                    # cross-engine semaphore, keeps the sync engine free
                    # for input prefetch)
                    nc.scalar.dma_start(
                        Ob[:, m, h0 * WO : (h0 + band_rows) * WO],
                        ot[:, m, :],
                    )

            # Input DMAs for the first bands go out before the (serially
            # issued) weight DMAs: the DVE needs band 0 well before the PE
            # needs the weights.
            PREFETCH = 3
            pending = []
            n_done = 0
            for j, job in enumerate(jobs):
                pending.append((job, emit_dma(job)))
                if j == 1:
                    emit_weight_dma()
                if len(pending) > PREFETCH:
                    pj, pit = pending.pop(0)
                    emit_compute(pj, pit, split=(n_done < 2 and pj[2] <= 8))
                    n_done += 1
            for pj, pit in pending:
                emit_compute(pj, pit, split=(n_done < 2 and pj[2] <= 8))
                n_done += 1
    nc.finalize()
    return nc


_MODULE = None


def _get_module():
    global _MODULE
    if _MODULE is None:
        _MODULE = build_module()
    return _MODULE


def run(I, w_c, w_s, w_b, trace=False, **trace_kwargs):
    import ml_dtypes

    I = np.ascontiguousarray(
        np.asarray(I, dtype=np.float32).astype(ml_dtypes.bfloat16)
    )
    w_c = np.asarray(w_c, dtype=np.float32)
    w_s = np.asarray(w_s, dtype=np.float32)
    wcp = np.ascontiguousarray((w_c - w_s).astype(ml_dtypes.bfloat16))
    ws16 = np.ascontiguousarray(w_s.astype(ml_dtypes.bfloat16))
    wb = np.ascontiguousarray(np.asarray(w_b), dtype=np.float32)

    nc = _get_module()
    in_maps = [
        {
            "I": I[c * IMG_PER_CORE : (c + 1) * IMG_PER_CORE],
            "wcp": wcp,
            "ws": ws16,
            "wb": wb,
        }
        for c in range(N_CORES)
    ]
    res = bass_utils.run_bass_kernel_spmd(
        nc, in_maps, core_ids=list(range(N_CORES)), trace=trace, **trace_kwargs
    )
    out = np.concatenate(
        [np.asarray(r["out"], dtype=np.float32) for r in res.results], axis=0
    )
    return out, res


def kernel(I, w_c, w_s, w_b):
    out, _ = run(I, w_c, w_s, w_b)
    return out


if __name__ == "__main__":
    rng = np.random.default_rng(0)
    I = rng.standard_normal((B, C_IN, H, W), dtype=np.float32)
    w_c = rng.standard_normal((C_IN, C_OUT), dtype=np.float32) * 0.0625
    w_s = rng.standard_normal((C_IN, C_OUT), dtype=np.float32) * 0.0078
    w_b = np.zeros((C_OUT,), dtype=np.float32)
    o = kernel(I=I, w_c=w_c, w_s=w_s, w_b=w_b)
    print("out", o.shape, o.dtype, float(np.abs(o).mean()))

